# revision 38
# baseline (speedup 1.0000x reference)
"""Multi-head attention (dense transformer block) on 8 Trainium2 NeuronCores.

Reference computation (per batch element b of 8):
    qkv = x @ w_qkv.T + b_qkv                  # [1024, 2304]
    q, k, v = split heads (12 heads, d=64)
    attn = softmax(q k^T / sqrt(d))
    out  = (attn v) reshaped @ w_proj.T + b_proj

Sharding: pure data parallel - core b handles batch element b, weights are
replicated, no collectives.

Bias algebra done host-side: b_k drops out of softmax entirely (row-constant
shift), b_v commutes through the normalized attention (rows sum to 1) so it
folds into a host-side constant w_proj @ b_v, and b_proj is added host-side.
Only b_q survives on-chip.

Per-core kernel (all matmul operands fp16, fp32 PSUM accumulation):
  C: v    = x Wv^T                  -> [1024, 12*128]: per head [v(64)|one(64)]
     so the PV matmul emits the softmax row-sum replicated on PSUM
     partitions 64:128 (no partition broadcast needed for the normalize)
  B: qk^T = [Wq*scale; Wk] x^T      -> [1536, 1024] (features on partitions)
  D: per head pair hp: scoresT = k^T q (two heads row-tiled into the two
     halves of the PE array), exp on ScalarE straight from PSUM pairs,
     PV accumulate, normalize by approx-reciprocal(rowsum).
     B tiles for wave hp+1 and PV of wave hp-1 are interleaved between the
     score matmuls so the PE never idles while ScalarE runs the exps.
  E: out = score w_proj^T, evicted fp16 (biases added host-side), DMA'd per
     o-half; wave 5 runs nq0 first so e-tiles 0-1 overlap its nq512 block.

All inputs are repacked host-side into the exact SBUF layouts; DMAs are
issued in consumption order with the first-wave operands (x0, x1, first wqk
slice) front-loaded so the exp stream starts as early as possible. Junk
warmup matmuls bridge the DMA-bound preamble so the PE HAM clock gate stays
at 2.4 GHz once lifted.
"""

import os
import sys

for _p in ("/opt/trn_rl_repo", "/root/.axon_site/_ro/trn_rl_repo"):
    if os.path.isdir(_p) and _p not in sys.path:
        sys.path.insert(0, _p)

import numpy as np

import concourse.bass as bass
import concourse.mybir as mybir
import concourse.tile as tile
from concourse import bacc
from concourse.bass_utils import run_bass_kernel_spmd

DIM = 768
N_HEAD = 12
HEAD_DIM = 64
SCALE = HEAD_DIM ** (-0.5)
NB = 8          # batch == number of cores
N = 1024        # sequence length
CCH = DIM // 128  # 6 contraction chunks

F32 = mybir.dt.float32
F16 = mybir.dt.float16
AF = mybir.ActivationFunctionType

# qk o-tiles (12 of 128) are laid out in SBUF in wave consumption order so
# the wqk DMA splits can land just in time.
OT_ORDER = [0, 6, 1, 7, 2, 8, 3, 9, 4, 10, 5, 11]
SLOT = {ot: s for s, ot in enumerate(OT_ORDER)}

_CACHE: dict = {}


def _build():
    nc = bacc.Bacc("TRN2", target_bir_lowering=False, debug=False)

    xp_d = nc.dram_tensor("xp", [128, 4, CCH, 256], F16, kind="ExternalInput")
    wqk_d = nc.dram_tensor("wqk_p", [128, CCH, 2 * DIM], F16, kind="ExternalInput")
    bqk_d = nc.dram_tensor("b_q_t", [128, 6], F32, kind="ExternalInput")
    wv_d = nc.dram_tensor("wv_p", [128, CCH, DIM], F16, kind="ExternalInput")
    wp_d = nc.dram_tensor("wp_p", [128, CCH, DIM], F16, kind="ExternalInput")
    out_d = nc.dram_tensor("out", [N, DIM], F16, kind="ExternalOutput")

    with tile.TileContext(nc) as tc:
        with (
            tc.tile_pool(name="consts", bufs=1) as consts,
            tc.tile_pool(name="qk", bufs=1) as qk_pool,
            tc.tile_pool(name="score", bufs=1) as score_pool,
            tc.tile_pool(name="v", bufs=1) as v_pool,
            tc.tile_pool(name="x", bufs=1) as x_pool,
            tc.tile_pool(name="wqk", bufs=1) as wqk_pool,
            tc.tile_pool(name="wv", bufs=1) as wv_pool,
            tc.tile_pool(name="wp", bufs=1) as wp_pool,
            tc.tile_pool(name="attn", bufs=32) as attn_pool,
            tc.tile_pool(name="small", bufs=4) as small_pool,
            tc.tile_pool(name="ostage", bufs=2) as out_pool,
            tc.tile_pool(name="ps", bufs=2, space="PSUM") as ps_pool,
            tc.tile_pool(name="pair", bufs=2, space="PSUM") as pair_pool,
            tc.tile_pool(name="acc", bufs=2, space="PSUM") as acc_pool,
        ):
            x_sb = x_pool.tile([128, 4, CCH, 256], F16)
            wv_sb = wv_pool.tile([128, CCH, DIM], F16)
            wqk_sb = wqk_pool.tile([128, CCH, 2 * DIM], F16)
            wp_sb = wp_pool.tile([128, CCH, DIM], F16)

            bqk_sb = consts.tile([128, 6], F32)

            # warmup junk operand: memset'd, costs ~100ns on the DVE
            warm_sb = consts.tile([128, 512], F16)
            nc.vector.memset(warm_sb[:], 0.0)

            # input DMAs in consumption order, first-wave operands first
            nc.sync.dma_start(x_sb[:, 0], xp_d[:, 0])
            nc.sync.dma_start(bqk_sb[:], bqk_d[:])
            nc.sync.dma_start(x_sb[:, 1], xp_d[:, 1])
            nc.sync.dma_start(wqk_sb[:, :, 0:256], wqk_d[:, :, 0:256])
            nc.sync.dma_start(x_sb[:, 2:4], xp_d[:, 2:4])
            nc.sync.dma_start(wv_sb[:, :, 0:512], wv_d[:, :, 0:512])
            nc.sync.dma_start(wqk_sb[:, :, 256:512], wqk_d[:, :, 256:512])
            nc.sync.dma_start(wv_sb[:, :, 512:DIM], wv_d[:, :, 512:DIM])
            nc.sync.dma_start(wqk_sb[:, :, 512:1024], wqk_d[:, :, 512:1024])
            nc.sync.dma_start(wqk_sb[:, :, 1024:1536], wqk_d[:, :, 1024:1536])
            nc.sync.dma_start(wp_sb[:], wp_d[:])

            def junk(n=1):
                # HAM-keeper junk matmuls: no input semaphores, so they fill
                # DMA-wait gaps and keep the PE clock gate lifted
                for _ in range(n):
                    ps = ps_pool.tile([128, 512], F32)
                    nc.tensor.matmul(
                        ps[:], warm_sb[:, 0:128], warm_sb[:], start=True, stop=True,
                    )

            qk_sb = qk_pool.tile([128, 12, N], F16)         # [o=1536, n]
            score_sb = score_pool.tile([128, CCH, N], F16)  # [c=768, n]
            v_sb = v_pool.tile([128, 8, N_HEAD * 128], F16)  # [n, h*(v|one64)]

            # ---- Phase C: v projection; the ones halves (cols 64:128 of
            # each head block) are memset per n-tile, interleaved into the
            # eviction gaps so the 10us-class flat memset never blocks the
            # critical b/c evictions on the in-order DVE ----
            v4 = v_sb[:].rearrange("p n (h d) -> p n h d", d=128)

            def ones_chunk(nt):
                nc.vector.memset(v4[:, nt, :, 0:64], 1.0)

            def c_group(nt, blk):
                o0, ow = ((0, 512), (512, 256))[blk]
                off = (0, 8 * 128)[blk]
                ps = ps_pool.tile([128, 512], F32)
                n0 = (nt % 2) * 128
                for c in range(CCH):
                    nc.tensor.matmul(
                        ps[:, :ow],
                        x_sb[:, nt // 2, c, n0:n0 + 128],
                        wv_sb[:, c, o0:o0 + ow],
                        start=(c == 0),
                        stop=(c == CCH - 1),
                    )
                nh = ow // 64
                src = ps[:, :ow].rearrange("p (h d) -> p h d", d=64)
                dst = v_sb[:, nt, off:off + nh * 128].rearrange(
                    "p (h d) -> p h d", d=128
                )[:, :, 64:128]
                nc.vector.tensor_copy(dst, src)

            # ---- Phase B helper: one [o-tile, nq] strip of the qk^T proj ----
            def b_group(ot, nq):
                sl = SLOT[ot]
                ps = ps_pool.tile([128, 512], F32)
                g0 = nq // 256
                for c in range(CCH):
                    nc.tensor.matmul(
                        ps[:],
                        wqk_sb[:, c, sl * 128:(sl + 1) * 128],
                        x_sb[:, g0:g0 + 2, c, :],
                        start=(c == 0),
                        stop=(c == CCH - 1),
                    )
                if ot < 6:
                    nc.vector.tensor_scalar_add(
                        qk_sb[:, ot, nq:nq + 512], ps[:], bqk_sb[:, ot:ot + 1],
                    )
                else:
                    nc.vector.tensor_copy(qk_sb[:, ot, nq:nq + 512], ps[:])

            # ---- Phase D helpers ----
            def score_pair(hp, nq, nk):
                """scoresT for both heads of pair hp, one nk tile: head A into
                cols 0:512 (PE rows 0-63), head B into 512:1024 (rows 64-127),
                then exp straight from the 2-bank PSUM pair into fp16 SBUF."""
                pair = pair_pool.tile([128, 1024], F32)
                for half, p0 in ((0, 0), (1, 64)):
                    nc.tensor.matmul(
                        pair[:, half * 512:(half + 1) * 512],
                        qk_sb[p0:p0 + 64, 6 + hp, nk * 128:(nk + 1) * 128],
                        qk_sb[p0:p0 + 64, hp, nq:nq + 512],
                        start=True, stop=True,
                        tile_position=(p0, 0),
                    )
                at = attn_pool.tile([128, 1024], F16)
                nc.scalar.activation(at[:], pair[:], AF.Exp)
                return at

            def pv_group(hp, nq, half, p0, attns, pool=None):
                """attn @ [v|ones] for one head/nq strip + normalize: the ones
                half replicates the rowsum on PSUM partitions 64:128, so the
                normalize is stage-copy + approx-reciprocal + multiply."""
                h = 2 * hp + half
                acc = (pool or acc_pool).tile([128, 512], F32, name="pair" if pool else "acc")
                for nk in range(8):
                    nc.tensor.matmul(
                        acc[:],
                        v_sb[:, nk, h * 128:(h + 1) * 128],
                        attns[nk][:, half * 512:(half + 1) * 512],
                        start=(nk == 0),
                        stop=(nk == 7),
                    )
                # ones occupy cols 0:64, so the replicated rowsums land on
                # PSUM partitions 0:64 where the custom reciprocal can read
                # them directly (the >0-offset misread doesn't apply) - no
                # staging copy needed
                rec = small_pool.tile([64, 512], F32, tag="rec")
                nc.vector.reciprocal_approx_fast(rec[:], acc[0:64, :])
                nc.vector.tensor_mul(
                    score_sb[p0:p0 + 64, hp, nq:nq + 512], acc[64:128, :], rec[:],
                )

            # ---- preamble + wave 0: the nq0 half needs only x0, x1 and the
            # first wqk slice; junk matmuls bridge the initial DMA wait ----
            junk(14)
            b_group(6, 0)
            b_group(0, 0)
            strips = {0: [], 512: []}
            # NOTE: a k-tile strip b(6+hp, nq) covers KEY tiles nk in
            # [nq/128, nq/128+4) - so b(6, 512) must be emitted before any
            # pair with nk >= 4. C-blk0 all runs here (pv of wave 1 needs
            # every nt).
            w0a_fillers = [
                lambda: b_group(6, 512),
                lambda: (c_group(0, 0), ones_chunk(0)),
                lambda: (c_group(1, 0), ones_chunk(1)),
                lambda: (c_group(2, 0), ones_chunk(2)),
                lambda: (c_group(3, 0), ones_chunk(3)),
                lambda: (c_group(4, 0), ones_chunk(4)),
                lambda: (c_group(5, 0), ones_chunk(5)),
                lambda: (c_group(6, 0), ones_chunk(6)),
            ]
            fi = 0
            for nk in range(8):
                strips[0].append(score_pair(0, 0, nk))
                if fi < len(w0a_fillers):
                    w0a_fillers[fi]()
                    fi += 1
            c_group(7, 0)
            ones_chunk(7)
            b_group(0, 512)
            w0b_fillers = [
                lambda: b_group(1, 0),
                lambda: b_group(1, 512),
                lambda: b_group(7, 0),
                lambda: b_group(7, 512),
            ]
            for nt in (0, 1, 2, 3):
                w0b_fillers.append(lambda nt=nt: c_group(nt, 1))
            fi = 0
            for nk in range(8):
                strips[512].append(score_pair(0, 512, nk))
                if fi < len(w0b_fillers):
                    w0b_fillers[fi]()
                    fi += 1
            while fi < len(w0b_fillers):
                w0b_fillers[fi]()
                fi += 1
            prev_strips = (0, strips)

            for hp in range(1, 5):
                # fillers keep the PE busy while ScalarE exps this wave
                bfill = []
                for ot in (hp + 1, 7 + hp):
                    for nq in (0, 512):
                        bfill.append(lambda ot=ot, nq=nq: b_group(ot, nq))
                if hp == 1:
                    for nt in (4, 5, 6, 7):
                        bfill.append(lambda nt=nt: c_group(nt, 1))
                pfill = []
                php, pstrips = prev_strips
                for nq in (0, 512):
                    for half, p0 in ((0, 0), (1, 64)):
                        pfill.append(
                            lambda nq=nq, half=half, p0=p0, php=php,
                                   s=pstrips: pv_group(php, nq, half, p0, s[nq])
                        )
                # alternate B and PV fillers: spacing the PV groups apart
                # lets each normalize chain drain before its PSUM acc bank
                # is recycled
                fillers = []
                for k in range(max(len(bfill), len(pfill))):
                    if k < len(bfill):
                        fillers.append(bfill[k])
                    if k < len(pfill):
                        fillers.append(pfill[k])
                strips = {0: [], 512: []}
                fi = 0
                for si, (nq, nk) in enumerate(
                    [(nq, nk) for nq in (0, 512) for nk in range(8)]
                ):
                    strips[nq].append(score_pair(hp, nq, nk))
                    if si % 2 == 1 and fi < len(fillers):
                        fillers[fi]()
                        fi += 1
                while fi < len(fillers):
                    fillers[fi]()
                    fi += 1
                prev_strips = (hp, strips)

            # ---- Phase E helper: copy eviction (biases added on the host),
            # fp16 staging, DMA per o-half ----
            def e_tile(nt, split=1, copy_eng=None):
                stage = out_pool.tile([128, DIM], F16)
                for o0, ow in ((0, 512), (512, 256)):
                    ps = ps_pool.tile([128, 512], F32)
                    for c in range(CCH):
                        nc.tensor.matmul(
                            ps[:, :ow],
                            score_sb[:, c, nt * 128:(nt + 1) * 128],
                            wp_sb[:, c, o0:o0 + ow],
                            start=(c == 0),
                            stop=(c == CCH - 1),
                        )
                    if copy_eng is nc.vector:
                        nc.vector.tensor_copy(stage[:, o0:o0 + ow], ps[:, :ow])
                    else:
                        nc.scalar.activation(
                            stage[:, o0:o0 + ow], ps[:, :ow], AF.Copy,
                        )
                    cw = ow // split
                    for j in range(split):
                        nc.sync.dma_start(
                            out_d[nt * 128:(nt + 1) * 128,
                                  o0 + j * cw:o0 + (j + 1) * cw],
                            stage[:, o0 + j * cw:o0 + (j + 1) * cw],
                        )

            # ---- wave 5: nq0 scores first so pv5(nq0) and the first e-tiles
            # overlap the nq512 score block; pv4 fills nq0 ----
            _, p4 = prev_strips
            strips5 = {0: [], 512: []}
            for nk in range(8):
                strips5[0].append(score_pair(5, 0, nk))
                if nk == 1:
                    pv_group(4, 0, 0, 0, p4[0])
                elif nk == 3:
                    pv_group(4, 0, 1, 64, p4[0])
                elif nk == 5:
                    pv_group(4, 512, 0, 0, p4[512])
                elif nk == 7:
                    pv_group(4, 512, 1, 64, p4[512])
            for nk in range(8):
                strips5[512].append(score_pair(5, 512, nk))
                if nk == 1:
                    pv_group(5, 0, 0, 0, strips5[0])
                elif nk == 3:
                    pv_group(5, 0, 1, 64, strips5[0])
            # tail: exps are done, ScalarE takes the evictions while the DVE
            # drains the last normalize chains; e-tiles trail their score
            # producers by several groups (LDWEIGHTS hoist margin)
            pv_group(5, 512, 0, 0, strips5[512], pool=pair_pool)
            e_tile(0, copy_eng=nc.vector)
            pv_group(5, 512, 1, 64, strips5[512], pool=pair_pool)
            e_tile(1, copy_eng=nc.vector)
            e_tile(2)
            e_tile(3)
            e_tile(4)
            e_tile(5)
            e_tile(6, split=2)
            e_tile(7, split=2, copy_eng=nc.vector)

    nc.compile()
    return nc


def _get_nc():
    if "nc" not in _CACHE:
        _CACHE["nc"] = _build()
    return _CACHE["nc"]


def _pack6(w):
    """[768, o] -> [128, 6, o] with pack[p, c, o] = w[128c+p, o]."""
    return np.ascontiguousarray(w.reshape(CCH, 128, -1).transpose(1, 0, 2))


def kernel(x, w_qkv, b_qkv, w_proj, b_proj, **run_kwargs):
    x = np.asarray(x, dtype=np.float32)
    w_qkv = np.asarray(w_qkv, dtype=np.float32)
    b_qkv = np.asarray(b_qkv, dtype=np.float32)
    w_proj = np.asarray(w_proj, dtype=np.float32)
    b_proj = np.asarray(b_proj, dtype=np.float32)

    # Host-side layout prep (no arithmetic beyond folding the 1/sqrt(d) scale
    # into the q projection and the v/proj biases into a final constant).
    w_qk = w_qkv[: 2 * DIM].copy()
    w_qk[:DIM] *= SCALE
    b_q = b_qkv[:DIM] * SCALE
    # wqk in wave-order o-slots: pack[p, c, 128*slot+j] = w_qk[128*ot+j, 128c+p]
    wq4 = w_qk.reshape(12, 128, DIM)[OT_ORDER]            # [slot, j, c-dim]
    wqk_p = np.ascontiguousarray(
        wq4.transpose(2, 0, 1).reshape(DIM, 2 * DIM)      # [c-dim, slot*128+j]
    )
    wqk_p = _pack6(wqk_p).astype(np.float16)              # [128, 6, 1536]
    b_q_t = np.ascontiguousarray(b_q.reshape(6, 128).T)   # [128, 6] f32
    wv_p = _pack6(np.ascontiguousarray(w_qkv[2 * DIM:].T)).astype(np.float16)
    wp_p = _pack6(np.ascontiguousarray(w_proj.T)).astype(np.float16)
    # b_k drops out of softmax; b_v rides through the normalized attention
    # into a constant: out += w_proj @ b_v + b_proj
    out_const = (w_proj @ b_qkv[2 * DIM:] + b_proj).astype(np.float32)

    nc = _get_nc()
    in_maps = []
    for b in range(NB):
        xp = _pack6(np.ascontiguousarray(x[b].T)).astype(np.float16)
        # [128, 6, 1024] -> [128, 4 n-groups, 6, 256] for 3KB DMA runs
        xp = np.ascontiguousarray(
            xp.reshape(128, CCH, 4, 256).transpose(0, 2, 1, 3))
        in_maps.append({
            "xp": xp,
            "wqk_p": wqk_p,
            "b_q_t": b_q_t,
            "wv_p": wv_p,
            "wp_p": wp_p,
        })
    res = run_bass_kernel_spmd(nc, in_maps, core_ids=list(range(NB)), **run_kwargs)
    out = np.stack(
        [res.results[b]["out"].astype(np.float32) for b in range(NB)], axis=0
    )
    out += out_const
    if run_kwargs:
        return out, res
    return out


if __name__ == "__main__":
    rng = np.random.default_rng(0)
    x = rng.standard_normal((NB, N, DIM), dtype=np.float32)
    w_qkv = rng.standard_normal((3 * DIM, DIM), dtype=np.float32) * DIM ** -0.5
    b_qkv = rng.standard_normal((3 * DIM,), dtype=np.float32) * 0.02
    w_proj = rng.standard_normal((DIM, DIM), dtype=np.float32) * DIM ** -0.5
    b_proj = rng.standard_normal((DIM,), dtype=np.float32) * 0.02
    out = kernel(x=x, w_qkv=w_qkv, b_qkv=b_qkv, w_proj=w_proj, b_proj=b_proj)
    print("out", out.shape, out.dtype, float(np.abs(out).mean()))


# revision 39
# speedup vs baseline: 1.0108x; 1.0108x over previous
"""Multi-head attention (dense transformer block) on 8 Trainium2 NeuronCores.

Reference computation (per batch element b of 8):
    qkv = x @ w_qkv.T + b_qkv                  # [1024, 2304]
    q, k, v = split heads (12 heads, d=64)
    attn = softmax(q k^T / sqrt(d))
    out  = (attn v) reshaped @ w_proj.T + b_proj

Sharding: pure data parallel - core b handles batch element b, weights are
replicated, no collectives.

Bias algebra done host-side: b_k drops out of softmax entirely (row-constant
shift), b_v commutes through the normalized attention (rows sum to 1) so it
folds into a host-side constant w_proj @ b_v, and b_proj is added host-side.
Only b_q survives on-chip.

Per-core kernel (all matmul operands fp16, fp32 PSUM accumulation):
  C: v    = x Wv^T                  -> [1024, 12*128]: per head [v(64)|one(64)]
     so the PV matmul emits the softmax row-sum replicated on PSUM
     partitions 64:128 (no partition broadcast needed for the normalize)
  B: qk^T = [Wq*scale; Wk] x^T      -> [1536, 1024] (features on partitions)
  D: per head pair hp: scoresT = k^T q (two heads row-tiled into the two
     halves of the PE array), exp on ScalarE straight from PSUM pairs,
     PV accumulate, normalize by approx-reciprocal(rowsum).
     B tiles for wave hp+1 and PV of wave hp-1 are interleaved between the
     score matmuls so the PE never idles while ScalarE runs the exps.
  E: out = score w_proj^T, evicted fp16 (biases added host-side), DMA'd per
     o-half; wave 5 runs nq0 first so e-tiles 0-1 overlap its nq512 block.

All inputs are repacked host-side into the exact SBUF layouts; DMAs are
issued in consumption order with the first-wave operands (x0, x1, first wqk
slice) front-loaded so the exp stream starts as early as possible. Junk
warmup matmuls bridge the DMA-bound preamble so the PE HAM clock gate stays
at 2.4 GHz once lifted.
"""

import os
import sys

for _p in ("/opt/trn_rl_repo", "/root/.axon_site/_ro/trn_rl_repo"):
    if os.path.isdir(_p) and _p not in sys.path:
        sys.path.insert(0, _p)

import numpy as np

import concourse.bass as bass
import concourse.mybir as mybir
import concourse.tile as tile
from concourse import bacc
from concourse.bass_utils import run_bass_kernel_spmd

DIM = 768
N_HEAD = 12
HEAD_DIM = 64
SCALE = HEAD_DIM ** (-0.5)
NB = 8          # batch == number of cores
N = 1024        # sequence length
CCH = DIM // 128  # 6 contraction chunks

F32 = mybir.dt.float32
F16 = mybir.dt.float16
AF = mybir.ActivationFunctionType

# qk o-tiles (12 of 128) are laid out in SBUF in wave consumption order so
# the wqk DMA splits can land just in time.
OT_ORDER = [0, 6, 1, 7, 2, 8, 3, 9, 4, 10, 5, 11]
SLOT = {ot: s for s, ot in enumerate(OT_ORDER)}

_CACHE: dict = {}


def _build():
    nc = bacc.Bacc("TRN2", target_bir_lowering=False, debug=False)

    xp_d = nc.dram_tensor("xp", [128, 4, CCH, 256], F16, kind="ExternalInput")
    wqk_d = nc.dram_tensor("wqk_p", [128, CCH, 2 * DIM], F16, kind="ExternalInput")
    bqk_d = nc.dram_tensor("b_q_t", [128, 6], F32, kind="ExternalInput")
    wv_d = nc.dram_tensor("wv_p", [128, CCH, DIM], F16, kind="ExternalInput")
    wp_d = nc.dram_tensor("wp_p", [128, CCH, DIM], F16, kind="ExternalInput")
    out_d = nc.dram_tensor("out", [N, DIM], F16, kind="ExternalOutput")

    with tile.TileContext(nc) as tc:
        with (
            tc.tile_pool(name="consts", bufs=1) as consts,
            tc.tile_pool(name="qk", bufs=1) as qk_pool,
            tc.tile_pool(name="score", bufs=1) as score_pool,
            tc.tile_pool(name="v", bufs=1) as v_pool,
            tc.tile_pool(name="x", bufs=1) as x_pool,
            tc.tile_pool(name="wqk", bufs=1) as wqk_pool,
            tc.tile_pool(name="wv", bufs=1) as wv_pool,
            tc.tile_pool(name="wp", bufs=1) as wp_pool,
            tc.tile_pool(name="attn", bufs=32) as attn_pool,
            tc.tile_pool(name="small", bufs=4) as small_pool,
            tc.tile_pool(name="ostage", bufs=2) as out_pool,
            tc.tile_pool(name="ps", bufs=2, space="PSUM") as ps_pool,
            tc.tile_pool(name="pair", bufs=2, space="PSUM") as pair_pool,
            tc.tile_pool(name="acc", bufs=2, space="PSUM") as acc_pool,
        ):
            x_sb = x_pool.tile([128, 4, CCH, 256], F16)
            wv_sb = wv_pool.tile([128, CCH, DIM], F16)
            wqk_sb = wqk_pool.tile([128, CCH, 2 * DIM], F16)
            wp_sb = wp_pool.tile([128, CCH, DIM], F16)

            bqk_sb = consts.tile([128, 6], F32)

            # warmup junk operand: memset'd, costs ~100ns on the DVE
            warm_sb = consts.tile([128, 512], F16)
            nc.vector.memset(warm_sb[:], 0.0)

            # input DMAs in consumption order, first-wave operands first
            nc.sync.dma_start(x_sb[:, 0], xp_d[:, 0])
            nc.sync.dma_start(bqk_sb[:], bqk_d[:])
            nc.sync.dma_start(x_sb[:, 1], xp_d[:, 1])
            nc.sync.dma_start(wqk_sb[:, :, 0:256], wqk_d[:, :, 0:256])
            nc.sync.dma_start(x_sb[:, 2:4], xp_d[:, 2:4])
            nc.sync.dma_start(wv_sb[:, :, 0:512], wv_d[:, :, 0:512])
            nc.sync.dma_start(wqk_sb[:, :, 256:512], wqk_d[:, :, 256:512])
            nc.sync.dma_start(wv_sb[:, :, 512:DIM], wv_d[:, :, 512:DIM])
            nc.sync.dma_start(wqk_sb[:, :, 512:1024], wqk_d[:, :, 512:1024])
            nc.sync.dma_start(wqk_sb[:, :, 1024:1536], wqk_d[:, :, 1024:1536])
            nc.sync.dma_start(wp_sb[:], wp_d[:])

            def junk(n=1):
                # HAM-keeper junk matmuls: no input semaphores, so they fill
                # DMA-wait gaps and keep the PE clock gate lifted
                for _ in range(n):
                    ps = ps_pool.tile([128, 512], F32)
                    nc.tensor.matmul(
                        ps[:], warm_sb[:, 0:128], warm_sb[:], start=True, stop=True,
                    )

            qk_sb = qk_pool.tile([128, 12, N], F16)         # [o=1536, n]
            score_sb = score_pool.tile([128, CCH, N], F16)  # [c=768, n]
            v_sb = v_pool.tile([128, 8, N_HEAD * 128], F16)  # [n, h*(v|one64)]

            # ---- Phase C: v projection; the ones halves (cols 64:128 of
            # each head block) are memset per n-tile, interleaved into the
            # eviction gaps so the 10us-class flat memset never blocks the
            # critical b/c evictions on the in-order DVE ----
            v4 = v_sb[:].rearrange("p n (h d) -> p n h d", d=128)

            def ones_chunk(nt):
                nc.vector.memset(v4[:, nt, :, 0:64], 1.0)

            def c_group(nt, blk):
                o0, ow = ((0, 512), (512, 256))[blk]
                off = (0, 8 * 128)[blk]
                ps = ps_pool.tile([128, 512], F32)
                n0 = (nt % 2) * 128
                for c in range(CCH):
                    nc.tensor.matmul(
                        ps[:, :ow],
                        x_sb[:, nt // 2, c, n0:n0 + 128],
                        wv_sb[:, c, o0:o0 + ow],
                        start=(c == 0),
                        stop=(c == CCH - 1),
                    )
                nh = ow // 64
                src = ps[:, :ow].rearrange("p (h d) -> p h d", d=64)
                dst = v_sb[:, nt, off:off + nh * 128].rearrange(
                    "p (h d) -> p h d", d=128
                )[:, :, 64:128]
                nc.vector.tensor_copy(dst, src)

            # ---- Phase B helper: one [o-tile, nq] strip of the qk^T proj ----
            def b_group(ot, nq):
                sl = SLOT[ot]
                ps = ps_pool.tile([128, 512], F32)
                g0 = nq // 256
                for c in range(CCH):
                    nc.tensor.matmul(
                        ps[:],
                        wqk_sb[:, c, sl * 128:(sl + 1) * 128],
                        x_sb[:, g0:g0 + 2, c, :],
                        start=(c == 0),
                        stop=(c == CCH - 1),
                    )
                if ot < 6:
                    nc.vector.tensor_scalar_add(
                        qk_sb[:, ot, nq:nq + 512], ps[:], bqk_sb[:, ot:ot + 1],
                    )
                else:
                    nc.vector.tensor_copy(qk_sb[:, ot, nq:nq + 512], ps[:])

            # ---- Phase D helpers ----
            def score_pair(hp, nq, nk):
                """scoresT for both heads of pair hp, one nk tile: head A into
                cols 0:512 (PE rows 0-63), head B into 512:1024 (rows 64-127),
                then exp straight from the 2-bank PSUM pair into fp16 SBUF."""
                pair = pair_pool.tile([128, 1024], F32)
                for half, p0 in ((0, 0), (1, 64)):
                    nc.tensor.matmul(
                        pair[:, half * 512:(half + 1) * 512],
                        qk_sb[p0:p0 + 64, 6 + hp, nk * 128:(nk + 1) * 128],
                        qk_sb[p0:p0 + 64, hp, nq:nq + 512],
                        start=True, stop=True,
                        tile_position=(p0, 0),
                    )
                at = attn_pool.tile([128, 1024], F16)
                nc.scalar.activation(at[:], pair[:], AF.Exp)
                return at

            def pv_group(hp, nq, half, p0, attns, pool=None):
                """attn @ [v|ones] for one head/nq strip + normalize: the ones
                half replicates the rowsum on PSUM partitions 64:128, so the
                normalize is stage-copy + approx-reciprocal + multiply."""
                h = 2 * hp + half
                acc = (pool or acc_pool).tile([128, 512], F32, name="pair" if pool else "acc")
                for nk in range(8):
                    nc.tensor.matmul(
                        acc[:],
                        v_sb[:, nk, h * 128:(h + 1) * 128],
                        attns[nk][:, half * 512:(half + 1) * 512],
                        start=(nk == 0),
                        stop=(nk == 7),
                    )
                # ones occupy cols 0:64, so the replicated rowsums land on
                # PSUM partitions 0:64 where the custom reciprocal can read
                # them directly (the >0-offset misread doesn't apply) - no
                # staging copy needed
                rec = small_pool.tile([64, 512], F32, tag="rec")
                nc.vector.reciprocal_approx_fast(rec[:], acc[0:64, :])
                nc.vector.tensor_mul(
                    score_sb[p0:p0 + 64, hp, nq:nq + 512], acc[64:128, :], rec[:],
                )

            # ---- preamble + wave 0: the nq0 half needs only x0, x1 and the
            # first wqk slice; junk matmuls bridge the initial DMA wait ----
            junk(16)
            b_group(6, 0)
            b_group(0, 0)
            strips = {0: [], 512: []}
            # NOTE: a k-tile strip b(6+hp, nq) covers KEY tiles nk in
            # [nq/128, nq/128+4) - so b(6, 512) must be emitted before any
            # pair with nk >= 4. C-blk0 all runs here (pv of wave 1 needs
            # every nt).
            w0a_fillers = [
                lambda: b_group(6, 512),
                lambda: (c_group(0, 0), ones_chunk(0)),
                lambda: (c_group(1, 0), ones_chunk(1)),
                lambda: (c_group(2, 0), ones_chunk(2)),
                lambda: (c_group(3, 0), ones_chunk(3)),
                lambda: (c_group(4, 0), ones_chunk(4)),
                lambda: (c_group(5, 0), ones_chunk(5)),
                lambda: (c_group(6, 0), ones_chunk(6)),
            ]
            fi = 0
            for nk in range(8):
                strips[0].append(score_pair(0, 0, nk))
                if fi < len(w0a_fillers):
                    w0a_fillers[fi]()
                    fi += 1
            c_group(7, 0)
            ones_chunk(7)
            b_group(0, 512)
            w0b_fillers = [
                lambda: b_group(1, 0),
                lambda: b_group(1, 512),
                lambda: b_group(7, 0),
                lambda: b_group(7, 512),
            ]
            for nt in (0, 1, 2, 3):
                w0b_fillers.append(lambda nt=nt: c_group(nt, 1))
            fi = 0
            for nk in range(8):
                strips[512].append(score_pair(0, 512, nk))
                if fi < len(w0b_fillers):
                    w0b_fillers[fi]()
                    fi += 1
            while fi < len(w0b_fillers):
                w0b_fillers[fi]()
                fi += 1
            prev_strips = (0, strips)

            for hp in range(1, 5):
                # fillers keep the PE busy while ScalarE exps this wave
                bfill = []
                for ot in (hp + 1, 7 + hp):
                    for nq in (0, 512):
                        bfill.append(lambda ot=ot, nq=nq: b_group(ot, nq))
                if hp == 1:
                    for nt in (4, 5, 6, 7):
                        bfill.append(lambda nt=nt: c_group(nt, 1))
                pfill = []
                php, pstrips = prev_strips
                for nq in (0, 512):
                    for half, p0 in ((0, 0), (1, 64)):
                        pfill.append(
                            lambda nq=nq, half=half, p0=p0, php=php,
                                   s=pstrips: pv_group(php, nq, half, p0, s[nq])
                        )
                # alternate B and PV fillers: spacing the PV groups apart
                # lets each normalize chain drain before its PSUM acc bank
                # is recycled
                fillers = []
                for k in range(max(len(bfill), len(pfill))):
                    if k < len(bfill):
                        fillers.append(bfill[k])
                    if k < len(pfill):
                        fillers.append(pfill[k])
                strips = {0: [], 512: []}
                fi = 0
                for si, (nq, nk) in enumerate(
                    [(nq, nk) for nq in (0, 512) for nk in range(8)]
                ):
                    strips[nq].append(score_pair(hp, nq, nk))
                    if si % 2 == 1 and fi < len(fillers):
                        fillers[fi]()
                        fi += 1
                while fi < len(fillers):
                    fillers[fi]()
                    fi += 1
                prev_strips = (hp, strips)

            # ---- Phase E helper: copy eviction (biases added on the host),
            # fp16 staging, DMA per o-half ----
            def e_tile(nt, split=1, copy_eng=None):
                stage = out_pool.tile([128, DIM], F16)
                for o0, ow in ((0, 512), (512, 256)):
                    ps = ps_pool.tile([128, 512], F32)
                    for c in range(CCH):
                        nc.tensor.matmul(
                            ps[:, :ow],
                            score_sb[:, c, nt * 128:(nt + 1) * 128],
                            wp_sb[:, c, o0:o0 + ow],
                            start=(c == 0),
                            stop=(c == CCH - 1),
                        )
                    if copy_eng is nc.vector:
                        nc.vector.tensor_copy(stage[:, o0:o0 + ow], ps[:, :ow])
                    else:
                        nc.scalar.activation(
                            stage[:, o0:o0 + ow], ps[:, :ow], AF.Copy,
                        )
                    cw = ow // split
                    for j in range(split):
                        nc.sync.dma_start(
                            out_d[nt * 128:(nt + 1) * 128,
                                  o0 + j * cw:o0 + (j + 1) * cw],
                            stage[:, o0 + j * cw:o0 + (j + 1) * cw],
                        )

            # ---- wave 5: nq0 scores first so pv5(nq0) and the first e-tiles
            # overlap the nq512 score block; pv4 fills nq0 ----
            _, p4 = prev_strips
            strips5 = {0: [], 512: []}
            for nk in range(8):
                strips5[0].append(score_pair(5, 0, nk))
                if nk == 1:
                    pv_group(4, 0, 0, 0, p4[0])
                elif nk == 3:
                    pv_group(4, 0, 1, 64, p4[0])
                elif nk == 5:
                    pv_group(4, 512, 0, 0, p4[512])
                elif nk == 7:
                    pv_group(4, 512, 1, 64, p4[512])
            for nk in range(8):
                strips5[512].append(score_pair(5, 512, nk))
                if nk == 1:
                    pv_group(5, 0, 0, 0, strips5[0])
                elif nk == 3:
                    pv_group(5, 0, 1, 64, strips5[0])
            # tail: exps are done, ScalarE takes the evictions while the DVE
            # drains the last normalize chains; e-tiles trail their score
            # producers by several groups (LDWEIGHTS hoist margin)
            pv_group(5, 512, 0, 0, strips5[512], pool=pair_pool)
            e_tile(0, copy_eng=nc.vector)
            pv_group(5, 512, 1, 64, strips5[512], pool=pair_pool)
            e_tile(1, copy_eng=nc.vector)
            e_tile(2)
            e_tile(3)
            e_tile(4)
            e_tile(5)
            e_tile(6, split=2)
            e_tile(7, split=2, copy_eng=nc.vector)

    nc.compile()
    return nc


def _get_nc():
    if "nc" not in _CACHE:
        _CACHE["nc"] = _build()
    return _CACHE["nc"]


def _pack6(w):
    """[768, o] -> [128, 6, o] with pack[p, c, o] = w[128c+p, o]."""
    return np.ascontiguousarray(w.reshape(CCH, 128, -1).transpose(1, 0, 2))


def kernel(x, w_qkv, b_qkv, w_proj, b_proj, **run_kwargs):
    x = np.asarray(x, dtype=np.float32)
    w_qkv = np.asarray(w_qkv, dtype=np.float32)
    b_qkv = np.asarray(b_qkv, dtype=np.float32)
    w_proj = np.asarray(w_proj, dtype=np.float32)
    b_proj = np.asarray(b_proj, dtype=np.float32)

    # Host-side layout prep (no arithmetic beyond folding the 1/sqrt(d) scale
    # into the q projection and the v/proj biases into a final constant).
    w_qk = w_qkv[: 2 * DIM].copy()
    w_qk[:DIM] *= SCALE
    b_q = b_qkv[:DIM] * SCALE
    # wqk in wave-order o-slots: pack[p, c, 128*slot+j] = w_qk[128*ot+j, 128c+p]
    wq4 = w_qk.reshape(12, 128, DIM)[OT_ORDER]            # [slot, j, c-dim]
    wqk_p = np.ascontiguousarray(
        wq4.transpose(2, 0, 1).reshape(DIM, 2 * DIM)      # [c-dim, slot*128+j]
    )
    wqk_p = _pack6(wqk_p).astype(np.float16)              # [128, 6, 1536]
    b_q_t = np.ascontiguousarray(b_q.reshape(6, 128).T)   # [128, 6] f32
    wv_p = _pack6(np.ascontiguousarray(w_qkv[2 * DIM:].T)).astype(np.float16)
    wp_p = _pack6(np.ascontiguousarray(w_proj.T)).astype(np.float16)
    # b_k drops out of softmax; b_v rides through the normalized attention
    # into a constant: out += w_proj @ b_v + b_proj
    out_const = (w_proj @ b_qkv[2 * DIM:] + b_proj).astype(np.float32)

    nc = _get_nc()
    in_maps = []
    for b in range(NB):
        xp = _pack6(np.ascontiguousarray(x[b].T)).astype(np.float16)
        # [128, 6, 1024] -> [128, 4 n-groups, 6, 256] for 3KB DMA runs
        xp = np.ascontiguousarray(
            xp.reshape(128, CCH, 4, 256).transpose(0, 2, 1, 3))
        in_maps.append({
            "xp": xp,
            "wqk_p": wqk_p,
            "b_q_t": b_q_t,
            "wv_p": wv_p,
            "wp_p": wp_p,
        })
    res = run_bass_kernel_spmd(nc, in_maps, core_ids=list(range(NB)), **run_kwargs)
    out = np.stack(
        [res.results[b]["out"].astype(np.float32) for b in range(NB)], axis=0
    )
    out += out_const
    if run_kwargs:
        return out, res
    return out


if __name__ == "__main__":
    rng = np.random.default_rng(0)
    x = rng.standard_normal((NB, N, DIM), dtype=np.float32)
    w_qkv = rng.standard_normal((3 * DIM, DIM), dtype=np.float32) * DIM ** -0.5
    b_qkv = rng.standard_normal((3 * DIM,), dtype=np.float32) * 0.02
    w_proj = rng.standard_normal((DIM, DIM), dtype=np.float32) * DIM ** -0.5
    b_proj = rng.standard_normal((DIM,), dtype=np.float32) * 0.02
    out = kernel(x=x, w_qkv=w_qkv, b_qkv=b_qkv, w_proj=w_proj, b_proj=b_proj)
    print("out", out.shape, out.dtype, float(np.abs(out).mean()))
